# revision 18
# baseline (speedup 1.0000x reference)
"""GQA attention kernel for Trainium2, 8 NeuronCores.

Sharding: core c -> (batch = c // 4, head-group g = c % 4).
Each core handles one batch and 8 contiguous Q heads (= 2 KV heads),
computes its slice of Q/K/V projections, RoPE, causal attention, and a
partial output projection (rows g*512:(g+1)*512 of Wo). Host sums the 4
partials per batch.

Layout trick: everything is computed transposed. Host passes x^T per
batch so projections run as W^T-stationary matmuls producing Q^T/K^T/V^T
([feature, token]) directly, which is exactly the layout the scores
matmul needs (contraction over head_dim on partitions). Scores are
computed transposed (S^T[j,i], keys on partitions) so the context
matmul can consume exp(S^T) directly with V as the stationary operand.
A ones-column appended to V yields the softmax denominators for free in
the same PSUM accumulation.

Perf structure: the attention runs ib-major (query blocks of 512) with
4 heads interleaved and scores issued one j-step ahead of the context
accumulation, so the PE never waits on the exp->mask chain. The output
projection for each ib is issued right after its heads finish, keeping
the PE stream dense. RoPE runs in bf16: a host-side permutation of the
head dims puts rotate-half partners in the same 32-partition quadrant so
the rotation is a single DVE stream_shuffle. Causal masking multiplies
only the 128-wide diagonal chunk by a single on-device triangle tile
(all other chunks are either fully kept or skipped). Softmax
normalization computes 1/denom as exp(-ln(denom)) on the scalar engine
(the denominator lands on PSUM partition 0 via a ones-column in Vaug),
then broadcasts across partitions on gpsimd.
"""

import sys
import math

for _p in ("/opt/trn_rl_repo",):
    if _p not in sys.path:
        sys.path.append(_p)

import numpy as np
import ml_dtypes

import concourse.bass as bass
from concourse import bacc
import concourse.mybir as mybir
import concourse.tile as tile
from concourse.bass_utils import run_bass_kernel_spmd
from concourse.masks import make_identity

BF16 = mybir.dt.bfloat16
F32 = mybir.dt.float32

B, S, D = 2, 2048, 2048
NH, NKV, HD = 32, 8, 64
GROUP = NH // NKV          # 4 q heads per kv head
NCORES = 8
CPB = NCORES // B          # 4 cores per batch
HPC = NH // CPB            # 8 q heads per core
KVPC = NKV // CPB          # 2 kv heads per core
QW = HPC * HD              # 512 projected q cols per core
KW = KVPC * HD             # 128 projected kv cols per core

NT = S // 128              # 16 seq tiles of 128
NBL = S // 512             # 4 seq blocks of 512
KT = D // 128              # 16 contraction tiles
QF = QW // 128             # 4 row-tiles of Q^T

# rope pairing: dims d and d+32 must live in the same 32-partition
# quadrant so rotate-half is a stream_shuffle. Row r holds dim DIMPERM[r].
DIMPERM64 = np.concatenate([
    np.arange(0, 16), np.arange(32, 48),
    np.arange(16, 32), np.arange(48, 64),
])
# in-quadrant shuffle: row i <- row i+16 (i<16) / i-16 (i>=16)
SHUF_MASK = [i + 16 for i in range(16)] + [i for i in range(16)]

_nc_cache = None


def _build():
    nc = bacc.Bacc()
    xT = nc.dram_tensor("xT", [D, S], BF16, kind="ExternalInput")
    wq = nc.dram_tensor("wq", [D, QW], BF16, kind="ExternalInput")
    wk = nc.dram_tensor("wk", [D, KW], BF16, kind="ExternalInput")
    wv = nc.dram_tensor("wv", [D, KW], BF16, kind="ExternalInput")
    wo = nc.dram_tensor("wo", [QW, D], BF16, kind="ExternalInput")
    cos2 = nc.dram_tensor("cos2", [128, S], BF16, kind="ExternalInput")
    sinm = nc.dram_tensor("sinm", [128, S], BF16, kind="ExternalInput")
    out = nc.dram_tensor("out", [S, D], BF16, kind="ExternalOutput")

    Exp = mybir.ActivationFunctionType.Exp

    with tile.TileContext(nc) as tc:
        with (
            tc.tile_pool(name="persist", bufs=1) as pp,
            tc.tile_pool(name="acc", bufs=4, space="PSUM") as accp,
            tc.tile_pool(name="stp", bufs=4, space="PSUM") as stp,
        ):
            # ---- persistent tiles ----
            Qb = [pp.tile([128, S], BF16, name=f"qb{f}", tag=f"qb{f}") for f in range(QF)]
            Kb = pp.tile([128, S], BF16, name="kb", tag="kb")
            Vaug = [pp.tile([128, NT, 128], BF16, name=f"vaug{k}", tag=f"vaug{k}") for k in range(KVPC)]
            ctxT = [pp.tile([128, S], BF16, name=f"ctxt{f}", tag=f"ctxt{f}") for f in range(QF)]
            tri = pp.tile([128, 128], BF16, name="tri", tag="tri")
            ident = pp.tile([128, 128], BF16, name="ident", tag="ident")

            make_identity(nc, ident[:])
            # tri[p, c] = 1 if p <= c else 0 (causal keep-mask for the
            # 128-wide diagonal chunk; all other chunks are never masked)
            nc.gpsimd.memset(tri[:], 1.0)
            nc.gpsimd.affine_select(
                out=tri[:], in_=tri[:], pattern=[[1, 128]],
                compare_op=mybir.AluOpType.is_ge, fill=0.0,
                base=0, channel_multiplier=-1)
            for k in range(KVPC):
                nc.gpsimd.memset(Vaug[k][:, :, 0:64], 0.0)
                nc.gpsimd.memset(Vaug[k][:, :, 0:1], 1.0)

            # ==== phase 1+2: load x/weights, projections + rope ====
            with (
                tc.tile_pool(name="proj", bufs=1) as jp,
                tc.tile_pool(name="rope", bufs=3) as rp,
            ):
                xt = [jp.tile([128, S], BF16, name=f"xt{k}", tag=f"xt{k}") for k in range(KT)]
                wqt = [jp.tile([128, QW], BF16, name=f"wqt{k}", tag=f"wqt{k}") for k in range(KT)]
                wkt = [jp.tile([128, KW], BF16, name=f"wkt{k}", tag=f"wkt{k}") for k in range(KT)]
                wvt = [jp.tile([128, KW], BF16, name=f"wvt{k}", tag=f"wvt{k}") for k in range(KT)]
                cos2t = jp.tile([128, S], BF16, name="cos2t", tag="cos2t")
                sinmt = jp.tile([128, S], BF16, name="sinmt", tag="sinmt")
                VtT = jp.tile([128, S], BF16, name="vtt", tag="vtt")

                for k in range(KT):
                    nc.sync.dma_start(xt[k][:], xT[k * 128:(k + 1) * 128, :])
                    nc.sync.dma_start(wkt[k][:], wk[k * 128:(k + 1) * 128, :])
                    nc.sync.dma_start(wvt[k][:], wv[k * 128:(k + 1) * 128, :])
                    nc.sync.dma_start(wqt[k][:], wq[k * 128:(k + 1) * 128, :])
                nc.sync.dma_start(cos2t[:], cos2[:, :])
                nc.sync.dma_start(sinmt[:], sinm[:, :])

                def rope_store(ps, dst, tcol):
                    # ps: psum [128, 512] f32 holding raw Q^T/K^T rows.
                    # dst[:, tcol:tcol+512] <- rope(ps) in bf16.
                    qf = rp.tile([128, 512], BF16, name="ropecp", tag="ropecp")
                    nc.scalar.copy(qf[:], ps[:])
                    rot = rp.tile([128, 512], BF16, name="roperot", tag="roperot")
                    nc.vector.stream_shuffle(rot[:], qf[:], SHUF_MASK)
                    a = rp.tile([128, 512], BF16, name="ropea", tag="ropea")
                    b = rp.tile([128, 512], BF16, name="ropeb", tag="ropeb")
                    nc.vector.tensor_mul(a[:], qf[:], cos2t[:, tcol:tcol + 512])
                    nc.vector.tensor_mul(b[:], rot[:], sinmt[:, tcol:tcol + 512])
                    nc.vector.tensor_add(dst[:, tcol:tcol + 512], a[:], b[:])

                # K^T: [128, S]
                for t in range(NBL):
                    ps = stp.tile([128, 512], F32, name="stbank", tag="stbank")
                    for k in range(KT):
                        nc.tensor.matmul(
                            ps[:], wkt[k][:], xt[k][:, t * 512:(t + 1) * 512],
                            start=(k == 0), stop=(k == KT - 1))
                    rope_store(ps, Kb, t * 512)

                # V^T: [128, S] (no rope)
                for t in range(NBL):
                    ps = stp.tile([128, 512], F32, name="stbank", tag="stbank")
                    for k in range(KT):
                        nc.tensor.matmul(
                            ps[:], wvt[k][:], xt[k][:, t * 512:(t + 1) * 512],
                            start=(k == 0), stop=(k == KT - 1))
                    nc.scalar.copy(VtT[:, t * 512:(t + 1) * 512], ps[:])

                # V_aug[kv][:, j, 64:128] = V^T[kv rows, j block].T ; col 0 = 1.0
                # ones in col 0 -> denominator on PSUM partition 0; V block at
                # col 64 because 64-partition reads must start at 0 or 64
                for kv in range(KVPC):
                    for j in range(NT):
                        tp = stp.tile([128, 512], BF16, name="stbank", tag="stbank")
                        kb = kv * 64
                        nc.tensor.transpose(
                            tp[0:128, 0:64],
                            VtT[kb:kb + 64, j * 128:(j + 1) * 128],
                            ident[kb:kb + 64, kb:kb + 64])
                        nc.scalar.copy(Vaug[kv][:, j, 64:128], tp[0:128, 0:64])

                # Q^T: rows f*128.. of [QW, S], t-major so ib=0 unblocks early
                for t in range(NBL):
                    for f in range(QF):
                        ps = stp.tile([128, 512], F32, name="stbank", tag="stbank")
                        for k in range(KT):
                            nc.tensor.matmul(
                                ps[:], wqt[k][:, f * 128:(f + 1) * 128],
                                xt[k][:, t * 512:(t + 1) * 512],
                                start=(k == 0), stop=(k == KT - 1))
                        rope_store(ps, Qb[f], t * 512)

            # ==== phase 3+4: attention (ib-major, 4-head groups) + out proj ====
            with (
                tc.tile_pool(name="attn", bufs=10) as ap,
                tc.tile_pool(name="smal", bufs=4) as sp,
                tc.tile_pool(name="nstage", bufs=8) as nst,
                tc.tile_pool(name="wout", bufs=1) as wp,
                tc.tile_pool(name="ostg", bufs=2) as op,
            ):
                wot = [wp.tile([128, D], BF16, name=f"wot{c}", tag=f"wot{c}") for c in range(QF)]
                for c in range(QF):
                    nc.sync.dma_start(wot[c][:], wo[c * 128:(c + 1) * 128, :])

                # Q tiles are head-permuted (host): tile f holds local
                # heads f (kv0, rows 0:64) and f+4 (kv1, rows 64:128), so
                # the K lhsT base partition always matches the Q rhs base.
                def scores(h, j, ib, pt_of):
                    kv, fq = h
                    qr = kv * 64
                    koff = j - 4 * ib
                    c0 = 128 * koff if koff > 0 else 0
                    st = stp.tile([128, 512], F32, name="stbank", tag="stbank")
                    nc.tensor.matmul(
                        st[:, c0:512],
                        Kb[kv * 64:(kv + 1) * 64, j * 128:(j + 1) * 128],
                        Qb[fq][qr:qr + 64, ib * 512 + c0:(ib + 1) * 512],
                        start=True, stop=True)
                    pt = ap.tile([128, 512], BF16, name="pt", tag="pt")
                    nc.scalar.activation(pt[:, c0:512], st[:, c0:512],
                                         Exp, scale=0.125)
                    if koff >= 0:
                        nc.vector.tensor_mul(pt[:, c0:c0 + 128],
                                             pt[:, c0:c0 + 128], tri[:])
                    pt_of[(h, j)] = pt

                def ctx(h, j, ib, cp, pt_of):
                    kv, fq = h
                    koff = j - 4 * ib
                    c0 = 128 * koff if koff > 0 else 0
                    pt = pt_of.pop((h, j))
                    nc.tensor.matmul(
                        cp[0:128, c0:512], Vaug[kv][:, j, :],
                        pt[:, c0:512],
                        start=(j == 0), stop=(j == 4 * ib + 3),
                        skip_group_check=True)

                def stage_out(cp):
                    # drain the finished accumulator to SBUF so the PSUM
                    # bank recycles in ~1us instead of waiting for the
                    # reciprocal chain; denom row via vector, ctx via scalar
                    dn = nst.tile([1, 512], F32, name="dn", tag="dn")
                    nc.vector.tensor_copy(dn[0:1, :], cp[0:1, :])
                    cs = nst.tile([64, 512], BF16, name="cs", tag="cs")
                    nc.scalar.copy(cs[0:64, :], cp[64:128, :])
                    return dn, cs

                def norm_fin(dn, cs, h, ib):
                    # all-SBUF normalize: bf16 reciprocal + broadcast + 2x mul
                    kv, fq = h
                    qr = kv * 64
                    rc = sp.tile([1, 512], BF16, name="rc", tag="rc")
                    with nc.allow_low_precision(reason="softmax denom recip in bf16"):
                        nc.vector.reciprocal(rc[0:1, :], dn[0:1, :])
                    bc = sp.tile([64, 512], BF16, name="bc", tag="bc")
                    nc.gpsimd.partition_broadcast(bc[0:64, :], rc[0:1, :])
                    nc.vector.tensor_mul(
                        ctxT[fq][qr:qr + 64, ib * 512:(ib + 1) * 512],
                        cs[0:64, :], bc[0:64, :])

                pending = []

                def flush_norms(n=None):
                    k = len(pending) if n is None else min(n, len(pending))
                    for _ in range(k):
                        pending.pop(0)()

                def outproj(ib):
                    for t in range(ib * 4, ib * 4 + 4):
                        ops = [accp.tile([128, 512], F32, name="accb", tag="accb")
                               for _ in range(NBL)]
                        for c in range(QF):
                            for o in range(NBL):
                                nc.tensor.matmul(
                                    ops[o][:],
                                    ctxT[c][:, t * 128:(t + 1) * 128],
                                    wot[c][:, o * 512:(o + 1) * 512],
                                    start=(c == 0), stop=(c == QF - 1))
                        ob = op.tile([128, D], BF16, name="ob", tag="ob")
                        for o in range(NBL):
                            if o % 2 == 0:
                                nc.scalar.copy(ob[:, o * 512:(o + 1) * 512],
                                               ops[o][:])
                            else:
                                nc.vector.tensor_copy(ob[:, o * 512:(o + 1) * 512],
                                                      ops[o][:])
                        nc.sync.dma_start(out[t * 128:(t + 1) * 128, :], ob[:])

                for ib in range(NBL):
                    jmax = 4 * ib + 3
                    for grp in range(2):
                        # 4 heads interleaved: (kv0,f), (kv1,f), (kv0,f'), (kv1,f')
                        heads = [(kv, fq) for fq in (2 * grp, 2 * grp + 1)
                                 for kv in range(KVPC)]
                        cp = {}
                        for h in heads:
                            cp[h] = accp.tile([128, 512], F32, name="accb", tag="accb")
                        pt_of = {}
                        for h in heads:
                            scores(h, 0, ib, pt_of)
                        for j in range(jmax + 1):
                            if j < jmax:
                                for h in heads:
                                    scores(h, j + 1, ib, pt_of)
                            if ib >= 1 and j >= 1:
                                # trickle deferred norms into mask-free
                                # j-steps so reciprocals never delay masks
                                flush_norms(1)
                            for h in heads:
                                ctx(h, j, ib, cp[h], pt_of)
                                if j == jmax:
                                    dn, cs = stage_out(cp[h])
                                    pending.append(
                                        lambda dn=dn, cs=cs, h=h, ib=ib:
                                        norm_fin(dn, cs, h, ib))
                        if grp == 0 and ib > 0:
                            flush_norms()
                            outproj(ib - 1)
                flush_norms()
                outproj(NBL - 1)

    nc.finalize()
    return nc


def _get_nc():
    global _nc_cache
    if _nc_cache is None:
        _nc_cache = _build()
    return _nc_cache


def _prep_inputs(x, cos, sin, Wq, Wk, Wv, Wo):
    bf = ml_dtypes.bfloat16
    dp = DIMPERM64
    cosT = cos.T.astype(np.float32)                                # [64, S]
    sinT = sin.T.astype(np.float32)
    sign = np.where(dp < 32, -1.0, 1.0)[:, None].astype(np.float32)
    cos64 = cosT[dp]                                               # [64, S]
    sinm64 = sign * sinT[dp]
    cos2 = np.ascontiguousarray(np.concatenate([cos64, cos64], 0)).astype(bf)
    sinm = np.ascontiguousarray(np.concatenate([sinm64, sinm64], 0)).astype(bf)
    # head permutation: Q^T tile f holds local heads (f, f+4) so that the
    # kv0/kv1 row base of K matches the q row base (PE base-partition rule).
    # Within each head the 64 dims are DIMPERM64-ordered (rope pairing).
    perm = [0, 4, 1, 5, 2, 6, 3, 7]
    colperm_q = np.concatenate([p * HD + dp for p in perm])        # [QW]
    colperm_o = np.concatenate([np.arange(HD) + p * HD for p in perm])
    colperm_k = np.concatenate([kv * HD + dp for kv in range(KVPC)])  # [KW]
    in_maps = []
    for c in range(NCORES):
        b, g = c // CPB, c % CPB
        xTb = np.ascontiguousarray(x[b].T.astype(bf))
        wq_g = Wq[:, g * QW:(g + 1) * QW][:, colperm_q]
        wk_g = Wk[:, g * KW:(g + 1) * KW][:, colperm_k]
        wo_g = Wo[g * QW:(g + 1) * QW, :][colperm_o, :]
        in_maps.append({
            "xT": xTb,
            "wq": np.ascontiguousarray(wq_g.astype(bf)),
            "wk": np.ascontiguousarray(wk_g.astype(bf)),
            "wv": np.ascontiguousarray(Wv[:, g * KW:(g + 1) * KW].astype(bf)),
            "wo": np.ascontiguousarray(wo_g.astype(bf)),
            "cos2": cos2,
            "sinm": sinm,
        })
    return in_maps


def kernel(x, mask, cos, sin, Wq, Wk, Wv, Wo, _trace=False, **kw):
    x = np.asarray(x, dtype=np.float32)
    in_maps = _prep_inputs(x, np.asarray(cos), np.asarray(sin),
                           np.asarray(Wq), np.asarray(Wk),
                           np.asarray(Wv), np.asarray(Wo))
    nc = _get_nc()
    res = run_bass_kernel_spmd(nc, in_maps, core_ids=list(range(NCORES)),
                               trace=_trace, **kw)
    parts = [np.asarray(r["out"], dtype=np.float32) for r in res.results]
    full = np.stack([
        sum(parts[b * CPB + g] for g in range(CPB)) for b in range(B)
    ]).astype(np.float32)
    if _trace:
        kernel.last_result = res
    return full


# revision 19
# speedup vs baseline: 1.2969x; 1.2969x over previous
"""GQA attention kernel for Trainium2, 8 NeuronCores.

Sharding: core c -> (batch = c // 4, head-group g = c % 4).
Each core handles one batch and 8 contiguous Q heads (= 2 KV heads),
computes its slice of Q/K/V projections, RoPE, causal attention, and a
partial output projection (rows g*512:(g+1)*512 of Wo). Host sums the 4
partials per batch.

Layout trick: everything is computed transposed. Host passes x^T per
batch so projections run as W^T-stationary matmuls producing Q^T/K^T/V^T
([feature, token]) directly, which is exactly the layout the scores
matmul needs (contraction over head_dim on partitions). Scores are
computed transposed (S^T[j,i], keys on partitions) so the context
matmul can consume exp(S^T) directly with V as the stationary operand.
A ones-column appended to V yields the softmax denominators for free in
the same PSUM accumulation.

Perf structure: the attention runs ib-major (query blocks of 512) with
4 heads interleaved and scores issued one j-step ahead of the context
accumulation, so the PE never waits on the exp->mask chain. The output
projection for each ib is issued right after its heads finish, keeping
the PE stream dense. RoPE runs in bf16: a host-side permutation of the
head dims puts rotate-half partners in the same 32-partition quadrant so
the rotation is a single DVE stream_shuffle. Causal masking multiplies
only the 128-wide diagonal chunk by a single on-device triangle tile
(all other chunks are either fully kept or skipped). Softmax
normalization computes 1/denom as exp(-ln(denom)) on the scalar engine
(the denominator lands on PSUM partition 0 via a ones-column in Vaug),
then broadcasts across partitions on gpsimd.
"""

import sys
import math

for _p in ("/opt/trn_rl_repo",):
    if _p not in sys.path:
        sys.path.append(_p)

import numpy as np
import ml_dtypes

import concourse.bass as bass
from concourse import bacc
import concourse.mybir as mybir
import concourse.tile as tile
from concourse.bass_utils import run_bass_kernel_spmd
from concourse.masks import make_identity

BF16 = mybir.dt.bfloat16
F32 = mybir.dt.float32

B, S, D = 2, 2048, 2048
NH, NKV, HD = 32, 8, 64
GROUP = NH // NKV          # 4 q heads per kv head
NCORES = 8
CPB = NCORES // B          # 4 cores per batch
HPC = NH // CPB            # 8 q heads per core
KVPC = NKV // CPB          # 2 kv heads per core
QW = HPC * HD              # 512 projected q cols per core
KW = KVPC * HD             # 128 projected kv cols per core

NT = S // 128              # 16 seq tiles of 128
NBL = S // 512             # 4 seq blocks of 512
KT = D // 128              # 16 contraction tiles
QF = QW // 128             # 4 row-tiles of Q^T

# rope pairing: dims d and d+32 must live in the same 32-partition
# quadrant so rotate-half is a stream_shuffle. Row r holds dim DIMPERM[r].
DIMPERM64 = np.concatenate([
    np.arange(0, 16), np.arange(32, 48),
    np.arange(16, 32), np.arange(48, 64),
])
# in-quadrant shuffle: row i <- row i+16 (i<16) / i-16 (i>=16)
SHUF_MASK = [i + 16 for i in range(16)] + [i for i in range(16)]

_nc_cache = None


def _build():
    nc = bacc.Bacc()
    xT = nc.dram_tensor("xT", [D, S], BF16, kind="ExternalInput")
    wq = nc.dram_tensor("wq", [D, QW], BF16, kind="ExternalInput")
    wk = nc.dram_tensor("wk", [D, KW], BF16, kind="ExternalInput")
    wv = nc.dram_tensor("wv", [D, KW], BF16, kind="ExternalInput")
    wo = nc.dram_tensor("wo", [QW, D], BF16, kind="ExternalInput")
    cos2 = nc.dram_tensor("cos2", [128, S], BF16, kind="ExternalInput")
    sinm = nc.dram_tensor("sinm", [128, S], BF16, kind="ExternalInput")
    out = nc.dram_tensor("out", [S, D], BF16, kind="ExternalOutput")

    Exp = mybir.ActivationFunctionType.Exp

    with tile.TileContext(nc) as tc:
        with (
            tc.tile_pool(name="persist", bufs=1) as pp,
            tc.tile_pool(name="acc", bufs=4, space="PSUM") as accp,
            tc.tile_pool(name="stp", bufs=4, space="PSUM") as stp,
        ):
            # ---- persistent tiles ----
            Qb = [pp.tile([128, S], BF16, name=f"qb{f}", tag=f"qb{f}") for f in range(QF)]
            Kb = pp.tile([128, S], BF16, name="kb", tag="kb")
            Vaug = [pp.tile([128, NT, 128], BF16, name=f"vaug{k}", tag=f"vaug{k}") for k in range(KVPC)]
            ctxT = [pp.tile([128, S], BF16, name=f"ctxt{f}", tag=f"ctxt{f}") for f in range(QF)]
            tri = pp.tile([128, 128], BF16, name="tri", tag="tri")
            ident = pp.tile([128, 128], BF16, name="ident", tag="ident")

            make_identity(nc, ident[:])
            # tri[p, c] = 1 if p <= c else 0 (causal keep-mask for the
            # 128-wide diagonal chunk; all other chunks are never masked)
            nc.gpsimd.memset(tri[:], 1.0)
            nc.gpsimd.affine_select(
                out=tri[:], in_=tri[:], pattern=[[1, 128]],
                compare_op=mybir.AluOpType.is_ge, fill=0.0,
                base=0, channel_multiplier=-1)
            for k in range(KVPC):
                nc.gpsimd.memset(Vaug[k][:, :, 0:64], 0.0)
                nc.gpsimd.memset(Vaug[k][:, :, 0:1], 1.0)

            # ==== phase 1+2: load x/weights, projections + rope ====
            with (
                tc.tile_pool(name="proj", bufs=1) as jp,
                tc.tile_pool(name="rope", bufs=3) as rp,
            ):
                xt = [jp.tile([128, S], BF16, name=f"xt{k}", tag=f"xt{k}") for k in range(KT)]
                wqt = [jp.tile([128, QW], BF16, name=f"wqt{k}", tag=f"wqt{k}") for k in range(KT)]
                wkt = [jp.tile([128, KW], BF16, name=f"wkt{k}", tag=f"wkt{k}") for k in range(KT)]
                wvt = [jp.tile([128, KW], BF16, name=f"wvt{k}", tag=f"wvt{k}") for k in range(KT)]
                cos2t = jp.tile([128, S], BF16, name="cos2t", tag="cos2t")
                sinmt = jp.tile([128, S], BF16, name="sinmt", tag="sinmt")
                VtT = jp.tile([128, S], BF16, name="vtt", tag="vtt")

                for k in range(KT):
                    nc.sync.dma_start(xt[k][:], xT[k * 128:(k + 1) * 128, :])
                    nc.sync.dma_start(wkt[k][:], wk[k * 128:(k + 1) * 128, :])
                    nc.sync.dma_start(wvt[k][:], wv[k * 128:(k + 1) * 128, :])
                    nc.sync.dma_start(wqt[k][:], wq[k * 128:(k + 1) * 128, :])
                nc.sync.dma_start(cos2t[:], cos2[:, :])
                nc.sync.dma_start(sinmt[:], sinm[:, :])

                def rope_store(ps, dst, tcol):
                    # ps: psum [128, 512] f32 holding raw Q^T/K^T rows.
                    # dst[:, tcol:tcol+512] <- rope(ps) in bf16.
                    qf = rp.tile([128, 512], BF16, name="ropecp", tag="ropecp")
                    nc.scalar.copy(qf[:], ps[:])
                    rot = rp.tile([128, 512], BF16, name="roperot", tag="roperot")
                    nc.vector.stream_shuffle(rot[:], qf[:], SHUF_MASK)
                    a = rp.tile([128, 512], BF16, name="ropea", tag="ropea")
                    b = rp.tile([128, 512], BF16, name="ropeb", tag="ropeb")
                    nc.vector.tensor_mul(a[:], qf[:], cos2t[:, tcol:tcol + 512])
                    nc.vector.tensor_mul(b[:], rot[:], sinmt[:, tcol:tcol + 512])
                    nc.vector.tensor_add(dst[:, tcol:tcol + 512], a[:], b[:])

                # K^T: [128, S]
                for t in range(NBL):
                    ps = stp.tile([128, 512], F32, name="stbank", tag="stbank")
                    for k in range(KT):
                        nc.tensor.matmul(
                            ps[:], wkt[k][:], xt[k][:, t * 512:(t + 1) * 512],
                            start=(k == 0), stop=(k == KT - 1))
                    rope_store(ps, Kb, t * 512)

                # V^T: [128, S] (no rope)
                for t in range(NBL):
                    ps = stp.tile([128, 512], F32, name="stbank", tag="stbank")
                    for k in range(KT):
                        nc.tensor.matmul(
                            ps[:], wvt[k][:], xt[k][:, t * 512:(t + 1) * 512],
                            start=(k == 0), stop=(k == KT - 1))
                    nc.scalar.copy(VtT[:, t * 512:(t + 1) * 512], ps[:])

                # V_aug[kv][:, j, 64:128] = V^T[kv rows, j block].T ; col 0 = 1.0
                # ones in col 0 -> denominator on PSUM partition 0; V block at
                # col 64 because 64-partition reads must start at 0 or 64
                for kv in range(KVPC):
                    for j in range(NT):
                        tp = stp.tile([128, 512], BF16, name="stbank", tag="stbank")
                        kb = kv * 64
                        nc.tensor.transpose(
                            tp[0:128, 0:64],
                            VtT[kb:kb + 64, j * 128:(j + 1) * 128],
                            ident[kb:kb + 64, kb:kb + 64])
                        nc.scalar.copy(Vaug[kv][:, j, 64:128], tp[0:128, 0:64])

                # Q^T: rows f*128.. of [QW, S], t-major so ib=0 unblocks early
                for t in range(NBL):
                    for f in range(QF):
                        ps = stp.tile([128, 512], F32, name="stbank", tag="stbank")
                        for k in range(KT):
                            nc.tensor.matmul(
                                ps[:], wqt[k][:, f * 128:(f + 1) * 128],
                                xt[k][:, t * 512:(t + 1) * 512],
                                start=(k == 0), stop=(k == KT - 1))
                        rope_store(ps, Qb[f], t * 512)

            # ==== phase 3+4: attention (ib-major, 4-head groups) + out proj ====
            with (
                tc.tile_pool(name="attn", bufs=10) as ap,
                tc.tile_pool(name="smal", bufs=4) as sp,
                tc.tile_pool(name="wout", bufs=1) as wp,
                tc.tile_pool(name="ostg", bufs=2) as op,
            ):
                wot = [wp.tile([128, D], BF16, name=f"wot{c}", tag=f"wot{c}") for c in range(QF)]
                for c in range(QF):
                    nc.sync.dma_start(wot[c][:], wo[c * 128:(c + 1) * 128, :])

                # Q tiles are head-permuted (host): tile f holds local
                # heads f (kv0, rows 0:64) and f+4 (kv1, rows 64:128), so
                # the K lhsT base partition always matches the Q rhs base.
                def scores(h, j, ib, pt_of):
                    kv, fq = h
                    qr = kv * 64
                    koff = j - 4 * ib
                    c0 = 128 * koff if koff > 0 else 0
                    st = stp.tile([128, 512], F32, name="stbank", tag="stbank")
                    nc.tensor.matmul(
                        st[:, c0:512],
                        Kb[kv * 64:(kv + 1) * 64, j * 128:(j + 1) * 128],
                        Qb[fq][qr:qr + 64, ib * 512 + c0:(ib + 1) * 512],
                        start=True, stop=True)
                    pt = ap.tile([128, 512], BF16, name="pt", tag="pt")
                    nc.scalar.activation(pt[:, c0:512], st[:, c0:512],
                                         Exp, scale=0.125)
                    if koff >= 0:
                        nc.vector.tensor_mul(pt[:, c0:c0 + 128],
                                             pt[:, c0:c0 + 128], tri[:])
                    pt_of[(h, j)] = pt

                def ctx(h, j, ib, cp, pt_of):
                    kv, fq = h
                    koff = j - 4 * ib
                    c0 = 128 * koff if koff > 0 else 0
                    pt = pt_of.pop((h, j))
                    nc.tensor.matmul(
                        cp[0:128, c0:512], Vaug[kv][:, j, :],
                        pt[:, c0:512],
                        start=(j == 0), stop=(j == 4 * ib + 3),
                        skip_group_check=True)

                def norm_ln(cp):
                    lnd = sp.tile([1, 512], F32, name="lnd", tag="lnd")
                    nc.scalar.activation(lnd[0:1, :], cp[0:1, :],
                                         mybir.ActivationFunctionType.Ln)
                    return lnd

                def norm_fin(cp, lnd, h, ib):
                    kv, fq = h
                    qr = kv * 64
                    rc = sp.tile([1, 512], F32, name="rc", tag="rc")
                    nc.scalar.activation(rc[0:1, :], lnd[0:1, :],
                                         mybir.ActivationFunctionType.Exp,
                                         scale=-1.0)
                    bc = sp.tile([64, 512], F32, name="bc", tag="bc")
                    nc.gpsimd.partition_broadcast(bc[0:64, :], rc[0:1, :])
                    nc.vector.tensor_mul(
                        ctxT[fq][qr:qr + 64, ib * 512:(ib + 1) * 512],
                        cp[64:128, :], bc[0:64, :])

                def outproj(ib):
                    for t in range(ib * 4, ib * 4 + 4):
                        ops = [accp.tile([128, 512], F32, name="accb", tag="accb")
                               for _ in range(NBL)]
                        for c in range(QF):
                            for o in range(NBL):
                                nc.tensor.matmul(
                                    ops[o][:],
                                    ctxT[c][:, t * 128:(t + 1) * 128],
                                    wot[c][:, o * 512:(o + 1) * 512],
                                    start=(c == 0), stop=(c == QF - 1))
                        ob = op.tile([128, D], BF16, name="ob", tag="ob")
                        for o in range(NBL):
                            if o % 2 == 0:
                                nc.scalar.copy(ob[:, o * 512:(o + 1) * 512],
                                               ops[o][:])
                            else:
                                nc.vector.tensor_copy(ob[:, o * 512:(o + 1) * 512],
                                                      ops[o][:])
                        nc.sync.dma_start(out[t * 128:(t + 1) * 128, :], ob[:])

                for ib in range(NBL):
                    jmax = 4 * ib + 3
                    for grp in range(2):
                        # 4 heads interleaved: (kv0,f), (kv1,f), (kv0,f'), (kv1,f')
                        heads = [(kv, fq) for fq in (2 * grp, 2 * grp + 1)
                                 for kv in range(KVPC)]
                        cp = {}
                        for h in heads:
                            cp[h] = accp.tile([128, 512], F32, name="accb", tag="accb")
                        pt_of = {}
                        for h in heads:
                            scores(h, 0, ib, pt_of)
                        for j in range(jmax + 1):
                            if j < jmax:
                                for h in heads:
                                    scores(h, j + 1, ib, pt_of)
                            for h in heads:
                                ctx(h, j, ib, cp[h], pt_of)
                        # cluster the Ln's then the Exp's so the act table
                        # switches twice per group instead of per head
                        lnd_of = {h: norm_ln(cp[h]) for h in heads}
                        for h in heads:
                            norm_fin(cp[h], lnd_of[h], h, ib)
                    outproj(ib)

    nc.finalize()
    return nc


def _get_nc():
    global _nc_cache
    if _nc_cache is None:
        _nc_cache = _build()
    return _nc_cache


def _prep_inputs(x, cos, sin, Wq, Wk, Wv, Wo):
    bf = ml_dtypes.bfloat16
    dp = DIMPERM64
    cosT = cos.T.astype(np.float32)                                # [64, S]
    sinT = sin.T.astype(np.float32)
    sign = np.where(dp < 32, -1.0, 1.0)[:, None].astype(np.float32)
    cos64 = cosT[dp]                                               # [64, S]
    sinm64 = sign * sinT[dp]
    cos2 = np.ascontiguousarray(np.concatenate([cos64, cos64], 0)).astype(bf)
    sinm = np.ascontiguousarray(np.concatenate([sinm64, sinm64], 0)).astype(bf)
    # head permutation: Q^T tile f holds local heads (f, f+4) so that the
    # kv0/kv1 row base of K matches the q row base (PE base-partition rule).
    # Within each head the 64 dims are DIMPERM64-ordered (rope pairing).
    perm = [0, 4, 1, 5, 2, 6, 3, 7]
    colperm_q = np.concatenate([p * HD + dp for p in perm])        # [QW]
    colperm_o = np.concatenate([np.arange(HD) + p * HD for p in perm])
    colperm_k = np.concatenate([kv * HD + dp for kv in range(KVPC)])  # [KW]
    in_maps = []
    for c in range(NCORES):
        b, g = c // CPB, c % CPB
        xTb = np.ascontiguousarray(x[b].T.astype(bf))
        wq_g = Wq[:, g * QW:(g + 1) * QW][:, colperm_q]
        wk_g = Wk[:, g * KW:(g + 1) * KW][:, colperm_k]
        wo_g = Wo[g * QW:(g + 1) * QW, :][colperm_o, :]
        in_maps.append({
            "xT": xTb,
            "wq": np.ascontiguousarray(wq_g.astype(bf)),
            "wk": np.ascontiguousarray(wk_g.astype(bf)),
            "wv": np.ascontiguousarray(Wv[:, g * KW:(g + 1) * KW].astype(bf)),
            "wo": np.ascontiguousarray(wo_g.astype(bf)),
            "cos2": cos2,
            "sinm": sinm,
        })
    return in_maps


def kernel(x, mask, cos, sin, Wq, Wk, Wv, Wo, _trace=False, **kw):
    x = np.asarray(x, dtype=np.float32)
    in_maps = _prep_inputs(x, np.asarray(cos), np.asarray(sin),
                           np.asarray(Wq), np.asarray(Wk),
                           np.asarray(Wv), np.asarray(Wo))
    nc = _get_nc()
    res = run_bass_kernel_spmd(nc, in_maps, core_ids=list(range(NCORES)),
                               trace=_trace, **kw)
    parts = [np.asarray(r["out"], dtype=np.float32) for r in res.results]
    full = np.stack([
        sum(parts[b * CPB + g] for g in range(CPB)) for b in range(B)
    ]).astype(np.float32)
    if _trace:
        kernel.last_result = res
    return full
